# revision 8
# baseline (speedup 1.0000x reference)
"""DiceLoss kernel for 8 Trainium2 NeuronCores.

Reference computation:
    inter[b,c] = sum_p pred[b,c,p] * target[b,c,p]          # [4, 8]
    denom      = sum(pred) + sum(target) + 1.0              # scalar
    loss_bc    = 2 * (inter + 1) / denom
    total      = sum_b( sum_c(loss_bc[b]) * 8**(b-4) ) / 4
    out        = 1 - total

Sharding: flatten (b, c) -> 32 rows of 2M pixels; core k takes rows
4k..4k+3 (contiguous => zero-copy host slices).  Each core streams its
64 MiB (pred+target) at DMA line rate with a raw-bass double-buffered
pipeline (this walrus build only accepts ONE sync-wait per instruction,
so all waits are standalone wait_ge ops):
  - DVE : fused tensor_tensor_reduce (product + free-axis accumulate)
          per chunk -> per-partition dot partials
  - ACT : activation(Copy, accum_out) per chunk -> sum(pred), sum(target)
  - PE  : one tiny matmul at the end folds the partition axis
Per-core output: [8, 3] f32 holding 4 row-dots + 2 global-sum partials.
Host combines 8 * 6 scalars into the final loss.
"""

from contextlib import ExitStack

import numpy as np

N, C, P = 4, 8, 2097152
NCORES = 8
ROWS = N * C                      # 32 (b,c) rows
RPC = ROWS // NCORES              # 4 rows per core
GROUP = 128 // RPC                # 32 partitions per row
FREE = RPC * P // 128             # 65536 free elems per partition
CHUNK = 4096
NCHUNK = FREE // CHUNK            # 16
BUFS = 4

_CACHE = {}


def _build_bass():
    import concourse.bass as bass
    import concourse.mybir as mybir

    f32 = mybir.dt.float32
    nc = bass.Bass("TRN2", target_bir_lowering=False, debug=False,
                   num_devices=NCORES)

    pred = nc.dram_tensor("pred", [RPC, P], f32, kind="ExternalInput").ap()
    targ = nc.dram_tensor("target", [RPC, P], f32, kind="ExternalInput").ap()
    gmat = nc.dram_tensor("gmat", [128, 8], f32, kind="ExternalInput").ap()
    out = nc.dram_tensor("out", [8, 3], f32, kind="ExternalOutput").ap()

    predf = pred.rearrange("r (g f) -> (r g) f", g=GROUP)
    targf = targ.rearrange("r (g f) -> (r g) f", g=GROUP)

    AX = mybir.AxisListType.X
    MUL = mybir.AluOpType.mult
    ADD = mybir.AluOpType.add
    COPY = mybir.ActivationFunctionType.Copy

    with ExitStack() as ctx:
        e = ctx.enter_context
        pred_sl = [e(nc.sbuf_tensor(f"pred_sl{i}", [128, CHUNK], f32))
                   for i in range(BUFS)]
        targ_sl = [e(nc.sbuf_tensor(f"targ_sl{i}", [128, CHUNK], f32))
                   for i in range(BUFS)]
        g_sb = e(nc.sbuf_tensor([128, 8], f32))
        dcols = e(nc.sbuf_tensor([128, NCHUNK], f32))
        pcols = e(nc.sbuf_tensor([128, NCHUNK], f32))
        tcols = e(nc.sbuf_tensor([128, NCHUNK], f32))
        dummy_v = e(nc.sbuf_tensor([128, NCHUNK], f32))
        dummy_a = e(nc.sbuf_tensor([128, 2 * NCHUNK], f32))
        fin = e(nc.sbuf_tensor([128, 3], f32))
        osb = e(nc.sbuf_tensor([8, 3], f32))
        ps = e(nc.psum_tensor([8, 3], f32))

        sg = e(nc.semaphore())   # gmat loaded
        # per-slot DMA-completion sems: in-flight DMAs on different slots
        # tick different sems, so a waiter knows exactly which load landed
        sp = [e(nc.semaphore(f"sp{i}")) for i in range(BUFS)]
        st = [e(nc.semaphore(f"st{i}")) for i in range(BUFS)]
        sv = e(nc.semaphore())   # DVE progress
        sa = e(nc.semaphore())   # ACT progress
        spe = e(nc.semaphore())  # PE matmul done
        so = e(nc.semaphore())   # osb ready

        block = e(nc.Block())

        @block.sync
        def _(sync):
            sync.dma_start(g_sb[:], gmat).then_inc(sg, 16)
            for k in range(NCHUNK):
                if k >= BUFS:
                    done = k - BUFS + 1
                    sync.wait_ge(sv, done)
                    sync.wait_ge(sa, 2 * done)
                s = k % BUFS
                sync.dma_start(
                    pred_sl[s][:], predf[:, k * CHUNK:(k + 1) * CHUNK]
                ).then_inc(sp[s], 16)
                sync.dma_start(
                    targ_sl[s][:], targf[:, k * CHUNK:(k + 1) * CHUNK]
                ).then_inc(st[s], 16)
            sync.wait_ge(so, 1)
            sync.dma_start(out, osb[:]).then_inc(sg, 16)

        @block.vector
        def _(vector):
            for k in range(NCHUNK):
                s = k % BUFS
                vector.wait_ge(sp[s], 16 * (k // BUFS + 1))
                vector.wait_ge(st[s], 16 * (k // BUFS + 1))
                # out = (pred * 1.0) * targ, accum_out = sum_f(out)
                nc.vector.scalar_tensor_tensor(
                    out=dummy_v[:, k:k + 1].broadcast_to((128, CHUNK)),
                    in0=pred_sl[s][:],
                    scalar=1.0,
                    in1=targ_sl[s][:],
                    op0=MUL,
                    op1=MUL,
                    accum_out=dcols[:, k:k + 1],
                ).then_inc(sv, 1)
            # epilogue: fold chunk columns, then partitions via PE
            vector.wait_ge(sv, NCHUNK)
            nc.vector.reduce_sum(fin[:, 0:1], dcols[:], axis=AX)
            vector.wait_ge(sa, 2 * NCHUNK)
            nc.vector.reduce_sum(fin[:, 1:2], pcols[:], axis=AX)
            nc.vector.reduce_sum(fin[:, 2:3], tcols[:], axis=AX).then_inc(sv, 1)
            vector.wait_ge(spe, 1)
            nc.vector.tensor_copy(osb[:], ps[:]).then_inc(so, 1)

        @block.scalar
        def _(scalar):
            for k in range(NCHUNK):
                s = k % BUFS
                scalar.wait_ge(sp[s], 16 * (k // BUFS + 1))
                nc.scalar.activation(
                    dummy_a[:, 2 * k:2 * k + 1].broadcast_to((128, CHUNK)),
                    pred_sl[s][:], COPY,
                    accum_out=pcols[:, k:k + 1],
                ).then_inc(sa, 1)
                scalar.wait_ge(st[s], 16 * (k // BUFS + 1))
                nc.scalar.activation(
                    dummy_a[:, 2 * k + 1:2 * k + 2].broadcast_to((128, CHUNK)),
                    targ_sl[s][:], COPY,
                    accum_out=tcols[:, k:k + 1],
                ).then_inc(sa, 1)

        @block.tensor
        def _(tensor):
            tensor.wait_ge(sg, 16)
            tensor.wait_ge(sv, NCHUNK + 1)
            nc.tensor.matmul(ps[:], g_sb[:], fin[:], start=True,
                             stop=True).then_inc(spe, 1)

    return nc


def _gmat() -> np.ndarray:
    g = np.zeros((128, 8), dtype=np.float32)
    g[np.arange(128), np.arange(128) // GROUP] = 1.0  # cols 0..3: row masks
    g[:, 4] = 1.0                                     # col 4: all-ones
    return g


def _make_in_maps(pred: np.ndarray, target: np.ndarray):
    predr = np.ascontiguousarray(pred, dtype=np.float32).reshape(ROWS, P)
    targr = np.ascontiguousarray(target, dtype=np.float32).reshape(ROWS, P)
    g = _gmat()
    return [
        {
            "pred": predr[k * RPC:(k + 1) * RPC],
            "target": targr[k * RPC:(k + 1) * RPC],
            "gmat": g,
        }
        for k in range(NCORES)
    ]


def _run(pred: np.ndarray, target: np.ndarray, trace: bool = False):
    from concourse.bass_utils import run_bass_kernel_spmd

    if "nc" not in _CACHE:
        _CACHE["nc"] = _build_bass()
    nc = _CACHE["nc"]
    in_maps = _make_in_maps(pred, target)
    return run_bass_kernel_spmd(nc, in_maps, core_ids=list(range(NCORES)),
                                trace=trace)


def _combine(results) -> np.ndarray:
    inter = np.empty(ROWS, dtype=np.float64)
    sp = 0.0
    st = 0.0
    for k in range(NCORES):
        o = np.asarray(results[k]["out"], dtype=np.float64)
        inter[k * RPC:(k + 1) * RPC] = o[0:RPC, 0]
        sp += o[4, 1]
        st += o[4, 2]
    denom = sp + st + 1.0
    loss_bc = 2.0 * (inter.reshape(N, C) + 1.0) / denom
    weights = np.float64(C) ** (np.arange(N, dtype=np.float64) - N)
    total = (loss_bc.sum(axis=1) * weights).sum() / N
    return np.array(1.0 - total, dtype=np.float32)


def kernel(pred: np.ndarray, target: np.ndarray) -> np.ndarray:
    pred = np.asarray(pred, dtype=np.float32)
    target = np.asarray(target, dtype=np.float32)
    res = _run(pred, target, trace=False)
    return _combine(res.results)


# revision 9
# speedup vs baseline: 1.1892x; 1.1892x over previous
"""DiceLoss kernel for 8 Trainium2 NeuronCores.

Reference computation:
    inter[b,c] = sum_p pred[b,c,p] * target[b,c,p]          # [4, 8]
    denom      = sum(pred) + sum(target) + 1.0              # scalar
    loss_bc    = 2 * (inter + 1) / denom
    total      = sum_b( sum_c(loss_bc[b]) * 8**(b-4) ) / 4
    out        = 1 - total

Sharding: flatten (b, c) -> 32 rows of 2M pixels; core k takes rows
4k..4k+3 (contiguous => zero-copy host slices).  Each core streams its
64 MiB (pred+target) at DMA line rate with a raw-bass double-buffered
pipeline (this walrus build only accepts ONE sync-wait per instruction,
so all waits are standalone wait_ge ops):
  - DVE : fused tensor_tensor_reduce (product + free-axis accumulate)
          per chunk -> per-partition dot partials
  - ACT : activation(Copy, accum_out) per chunk -> sum(pred), sum(target)
  - PE  : one tiny matmul at the end folds the partition axis
Per-core output: [8, 3] f32 holding 4 row-dots + 2 global-sum partials.
Host combines 8 * 6 scalars into the final loss.
"""

from contextlib import ExitStack

import numpy as np

N, C, P = 4, 8, 2097152
NCORES = 8
ROWS = N * C                      # 32 (b,c) rows
RPC = ROWS // NCORES              # 4 rows per core
GROUP = 128 // RPC                # 32 partitions per row
FREE = RPC * P // 128             # 65536 free elems per partition
CHUNK = 4096
NCHUNK = FREE // CHUNK            # 16
BUFS = 5
TAIL_SPLIT = 4                    # last chunk in 4 pieces -> shorter tail
# (offset, width) load pieces; tail pieces shrink the post-last-load latency
PIECES = [(k * CHUNK, CHUNK) for k in range(NCHUNK - 1)]
PIECES += [((NCHUNK - 1) * CHUNK + j * (CHUNK // TAIL_SPLIT),
            CHUNK // TAIL_SPLIT) for j in range(TAIL_SPLIT)]
NP = len(PIECES)                  # 19

_CACHE = {}


def _build_bass():
    import concourse.bass as bass
    import concourse.mybir as mybir

    f32 = mybir.dt.float32
    nc = bass.Bass("TRN2", target_bir_lowering=False, debug=False,
                   num_devices=NCORES)

    pred = nc.dram_tensor("pred", [RPC, P], f32, kind="ExternalInput").ap()
    targ = nc.dram_tensor("target", [RPC, P], f32, kind="ExternalInput").ap()
    gmat = nc.dram_tensor("gmat", [128, 8], f32, kind="ExternalInput").ap()
    out = nc.dram_tensor("out", [8, 3], f32, kind="ExternalOutput").ap()

    predf = pred.rearrange("r (g f) -> (r g) f", g=GROUP)
    targf = targ.rearrange("r (g f) -> (r g) f", g=GROUP)

    AX = mybir.AxisListType.X
    MUL = mybir.AluOpType.mult
    ADD = mybir.AluOpType.add
    COPY = mybir.ActivationFunctionType.Copy

    with ExitStack() as ctx:
        e = ctx.enter_context
        pred_sl = [e(nc.sbuf_tensor(f"pred_sl{i}", [128, CHUNK], f32))
                   for i in range(BUFS)]
        targ_sl = [e(nc.sbuf_tensor(f"targ_sl{i}", [128, CHUNK], f32))
                   for i in range(BUFS)]
        g_sb = e(nc.sbuf_tensor([128, 8], f32))
        dcols = e(nc.sbuf_tensor([128, NP], f32))
        pcols = e(nc.sbuf_tensor([128, NP], f32))
        tcols = e(nc.sbuf_tensor([128, NP], f32))
        dummy_v = e(nc.sbuf_tensor([128, NP], f32))
        dummy_a = e(nc.sbuf_tensor([128, 2 * NP], f32))
        fin = e(nc.sbuf_tensor([128, 3], f32))
        osb = e(nc.sbuf_tensor([8, 3], f32))
        ps = e(nc.psum_tensor([8, 3], f32))

        sg = e(nc.semaphore())   # gmat loaded
        # per-slot DMA-completion sems: in-flight DMAs on different slots
        # tick different sems, so a waiter knows exactly which load landed
        sp = [e(nc.semaphore(f"sp{i}")) for i in range(BUFS)]
        st = [e(nc.semaphore(f"st{i}")) for i in range(BUFS)]
        sv = e(nc.semaphore())   # DVE progress
        sa = e(nc.semaphore())   # ACT progress
        spe = e(nc.semaphore())  # PE matmul done
        so = e(nc.semaphore())   # osb ready

        block = e(nc.Block(no_gpsimd_drain=True))

        @block.sync
        def _(sync):
            for i, (off, w) in enumerate(PIECES):
                if i >= BUFS:
                    done = i - BUFS + 1
                    sync.wait_ge(sv, done)
                    sync.wait_ge(sa, 2 * done)
                s = i % BUFS
                sync.dma_start(
                    pred_sl[s][:, 0:w], predf[:, off:off + w]
                ).then_inc(sp[s], 16)
                sync.dma_start(
                    targ_sl[s][:, 0:w], targf[:, off:off + w]
                ).then_inc(st[s], 16)
            sync.wait_ge(so, 1)
            sync.dma_start(out, osb[:]).then_inc(sg, 16)

        @block.vector
        def _(vector):
            for i, (off, w) in enumerate(PIECES):
                s = i % BUFS
                use = 16 * (i // BUFS + 1)
                vector.wait_ge(sp[s], use)
                vector.wait_ge(st[s], use)
                # out = (pred * 1.0) * targ, accum_out = sum_f(out)
                nc.vector.scalar_tensor_tensor(
                    out=dummy_v[:, i:i + 1].broadcast_to((128, w)),
                    in0=pred_sl[s][:, 0:w],
                    scalar=1.0,
                    in1=targ_sl[s][:, 0:w],
                    op0=MUL,
                    op1=MUL,
                    accum_out=dcols[:, i:i + 1],
                ).then_inc(sv, 1)
            # epilogue: fold piece columns, then partitions via PE
            vector.wait_ge(sv, NP)
            nc.vector.reduce_sum(fin[:, 0:1], dcols[:], axis=AX)
            vector.wait_ge(sa, 2 * NP)
            nc.vector.reduce_sum(fin[:, 1:2], pcols[:], axis=AX)
            nc.vector.reduce_sum(fin[:, 2:3], tcols[:], axis=AX).then_inc(sv, 1)
            vector.wait_ge(spe, 1)
            nc.vector.tensor_copy(osb[:], ps[:]).then_inc(so, 1)

        @block.scalar
        def _(scalar):
            # gmat load rides the idle ACT HWDGE ring, off the SP stream
            scalar.dma_start(g_sb[:], gmat).then_inc(sg, 16)
            for i, (off, w) in enumerate(PIECES):
                s = i % BUFS
                use = 16 * (i // BUFS + 1)
                scalar.wait_ge(sp[s], use)
                nc.scalar.activation(
                    dummy_a[:, 2 * i:2 * i + 1].broadcast_to((128, w)),
                    pred_sl[s][:, 0:w], COPY,
                    accum_out=pcols[:, i:i + 1],
                ).then_inc(sa, 1)
                scalar.wait_ge(st[s], use)
                nc.scalar.activation(
                    dummy_a[:, 2 * i + 1:2 * i + 2].broadcast_to((128, w)),
                    targ_sl[s][:, 0:w], COPY,
                    accum_out=tcols[:, i:i + 1],
                ).then_inc(sa, 1)

        @block.tensor
        def _(tensor):
            tensor.wait_ge(sg, 16)
            tensor.wait_ge(sv, NP + 1)
            nc.tensor.matmul(ps[:], g_sb[:], fin[:], start=True,
                             stop=True).then_inc(spe, 1)

    return nc


def _gmat() -> np.ndarray:
    g = np.zeros((128, 8), dtype=np.float32)
    g[np.arange(128), np.arange(128) // GROUP] = 1.0  # cols 0..3: row masks
    g[:, 4] = 1.0                                     # col 4: all-ones
    return g


def _make_in_maps(pred: np.ndarray, target: np.ndarray):
    predr = np.ascontiguousarray(pred, dtype=np.float32).reshape(ROWS, P)
    targr = np.ascontiguousarray(target, dtype=np.float32).reshape(ROWS, P)
    g = _gmat()
    return [
        {
            "pred": predr[k * RPC:(k + 1) * RPC],
            "target": targr[k * RPC:(k + 1) * RPC],
            "gmat": g,
        }
        for k in range(NCORES)
    ]


def _run(pred: np.ndarray, target: np.ndarray, trace: bool = False):
    from concourse.bass_utils import run_bass_kernel_spmd

    if "nc" not in _CACHE:
        _CACHE["nc"] = _build_bass()
    nc = _CACHE["nc"]
    in_maps = _make_in_maps(pred, target)
    return run_bass_kernel_spmd(nc, in_maps, core_ids=list(range(NCORES)),
                                trace=trace)


def _combine(results) -> np.ndarray:
    inter = np.empty(ROWS, dtype=np.float64)
    sp = 0.0
    st = 0.0
    for k in range(NCORES):
        o = np.asarray(results[k]["out"], dtype=np.float64)
        inter[k * RPC:(k + 1) * RPC] = o[0:RPC, 0]
        sp += o[4, 1]
        st += o[4, 2]
    denom = sp + st + 1.0
    loss_bc = 2.0 * (inter.reshape(N, C) + 1.0) / denom
    weights = np.float64(C) ** (np.arange(N, dtype=np.float64) - N)
    total = (loss_bc.sum(axis=1) * weights).sum() / N
    return np.array(1.0 - total, dtype=np.float32)


def kernel(pred: np.ndarray, target: np.ndarray) -> np.ndarray:
    pred = np.asarray(pred, dtype=np.float32)
    target = np.asarray(target, dtype=np.float32)
    res = _run(pred, target, trace=False)
    return _combine(res.results)
